# revision 1
# baseline (speedup 1.0000x reference)
"""Bass/Tile MHA kernel for trn2, sharded over 8 cores as (batch, head-group).

Each core handles one batch b and 3 heads. Inputs are host-prepared:
  qt, kt, vt : [D, S] fp32   — Q[b].T etc. (transposed on host)
  mt         : [S, S] bf16   — mask[b,0].T  (mt[k, q] = mask[b,0,q,k]), 0/1
  wqt, wkt, wvt : [D, 3*DK] fp32 — W_X.T[:, head_cols]
  wot        : [3*DK, D] fp32    — W_O.T[head_rows, :]
Output:
  out : [S, D] fp32 — partial output (sum over the 4 head-groups of a batch
        gives the final output rows for that batch).
"""

import numpy as np

import concourse.bass as bass
import concourse.bacc as bacc
import concourse.tile as tile
import concourse.mybir as mybir

F32 = mybir.dt.float32
F32R = mybir.dt.float32r
BF16 = mybir.dt.bfloat16
AF = mybir.ActivationFunctionType
ALU = mybir.AluOpType

D = 768
DK = 64
NH = 3          # heads per core
HD = NH * DK    # 192


def build_mha_nc(S=2048, n_cores=8, norm_via_dram=True, reps=1):
    ST = S // 128   # s-tiles (also attention k-tiles)
    QQ = S // 512   # q quarters
    KT6 = D // 128  # contraction tiles for projections

    nc = bacc.Bacc("TRN2", target_bir_lowering=False, debug=False,
                   num_devices=n_cores)

    qt_d = nc.dram_tensor("qt", [D, S], F32R, kind="ExternalInput")
    kt_d = nc.dram_tensor("kt", [D, S], F32R, kind="ExternalInput")
    vt_d = nc.dram_tensor("vt", [D, S], F32R, kind="ExternalInput")
    mt_d = nc.dram_tensor("mt", [S, S], BF16, kind="ExternalInput")
    wqt_d = nc.dram_tensor("wqt", [D, HD], F32R, kind="ExternalInput")
    wkt_d = nc.dram_tensor("wkt", [D, HD], F32R, kind="ExternalInput")
    wvt_d = nc.dram_tensor("wvt", [D, HD], F32R, kind="ExternalInput")
    wot_d = nc.dram_tensor("wot", [HD, D], F32R, kind="ExternalInput")
    out_d = nc.dram_tensor("out", [S, D], F32, kind="ExternalOutput")
    scratch_d = nc.dram_tensor("recip_scratch", [NH, S], F32)

    with tile.TileContext(nc) as tc:
      for _rep in range(reps):
        with tc.tile_pool(name="perm", bufs=1) as perm:
          with (
            tc.tile_pool(name="weights", bufs=1) as wpool,
            tc.tile_pool(name="raw", bufs=2) as raw_pool,
            tc.tile_pool(name="psum_proj", bufs=3, space="PSUM") as psum_proj,
            tc.tile_pool(name="psum_v", bufs=2, space="PSUM") as psum_v_pool,
          ):
            # ---- persistent SBUF tensors ----
            qT_a = perm.tile([128, S], F32R, tag="qT_a")   # h0 rows 0-63, h1 rows 64-127
            qT_b = perm.tile([64, S], F32R, tag="qT_b")    # h2
            kT_a = perm.tile([128, S], F32R, tag="kT_a")
            kT_b = perm.tile([64, S], F32R, tag="kT_b")
            v_sb = perm.tile([128, ST, NH, DK + 1], BF16, tag="v_sb")
            attnT_a = perm.tile([128, S], F32R, tag="attnT_a")
            attnT_b = perm.tile([64, S], F32R, tag="attnT_b")

            nc.vector.memset(v_sb[:], 1.0)

            # ---- phase 1: projections ----
            wq_sb = wpool.tile([128, KT6, HD], F32R, tag="wq")
            wk_sb = wpool.tile([128, KT6, HD], F32R, tag="wk")
            wv_sb = wpool.tile([128, KT6, HD], F32R, tag="wv")
            nc.sync.dma_start(wq_sb[:], wqt_d.ap().rearrange("(o p) m -> p o m", p=128))
            nc.sync.dma_start(wk_sb[:], wkt_d.ap().rearrange("(o p) m -> p o m", p=128))
            nc.sync.dma_start(wv_sb[:], wvt_d.ap().rearrange("(o p) m -> p o m", p=128))

            def load_raw(x_dram, name):
                # per-kt DMAs: finer deps, more queue parallelism
                x_raw = raw_pool.tile([128, KT6, S], F32R, tag="raw", name=name)
                x_t = x_dram.ap().rearrange("(o p) s -> p o s", p=128)
                for kt in range(KT6):
                    nc.sync.dma_start(x_raw[:, kt, :], x_t[:, kt, :])
                return x_raw

            def project_T(x_raw, w_sb, dst_a, dst_b):
                # dst_a[0:128] = (x @ w[:, 0:128]).T ; dst_b[0:64] = (x @ w[:, 128:192]).T
                PW = min(1024, S)
                for mt_i, (dst, mw) in enumerate([(dst_a, 128), (dst_b, 64)]):
                    for w in range(S // PW):
                        ps = psum_proj.tile([128, PW], F32, tag="ps_proj",
                                            name="ps_proj")
                        for kt in range(KT6):
                            for half in range(PW // 512):
                                nc.tensor.matmul(
                                    ps[:mw, half * 512:(half + 1) * 512],
                                    w_sb[:, kt, mt_i * 128: mt_i * 128 + mw]
                                    ,
                                    x_raw[:, kt, w * PW + half * 512:
                                          w * PW + (half + 1) * 512],
                                    start=(kt == 0), stop=(kt == KT6 - 1))
                        nc.any.tensor_copy(
                            dst[:mw, w * PW:(w + 1) * PW], ps[:mw, :])

            q_raw = load_raw(qt_d, "q_raw")
            project_T(q_raw, wq_sb, qT_a, qT_b)
            k_raw = load_raw(kt_d, "k_raw")
            project_T(k_raw, wk_sb, kT_a, kT_b)

            # v projection: v[s, 3*64] in s-major layout, cast to bf16
            v_raw = load_raw(vt_d, "v_raw")
            for st in range(ST):
                psv = psum_v_pool.tile([128, HD], F32, tag="psv")
                for kt in range(KT6):
                    nc.tensor.matmul(
                        psv[:],
                        v_raw[:, kt, st * 128:(st + 1) * 128],
                        wv_sb[:, kt, :],
                        start=(kt == 0), stop=(kt == KT6 - 1))
                nc.any.tensor_copy(
                    v_sb[:, st, :, 0:DK],
                    psv[:].rearrange("p (h d) -> p h d", h=NH))

          # ---- phase 2: attention (+ interleaved output projection) ----
          with (
            tc.tile_pool(name="mask", bufs=6) as mask_pool,
            tc.tile_pool(name="expp", bufs=6) as exp_pool,
            tc.tile_pool(name="expm", bufs=6) as expm_pool,
            tc.tile_pool(name="ps_s", bufs=3, space="PSUM") as psum_s_pool,
            tc.tile_pool(name="ps_av", bufs=1, space="PSUM") as psum_av_pool,
            tc.tile_pool(name="norm", bufs=4) as norm_pool,
            tc.tile_pool(name="wo", bufs=1) as wo_pool,
            tc.tile_pool(name="outp", bufs=3) as out_pool,
            tc.tile_pool(name="ps_o", bufs=1, space="PSUM") as psum_o_pool,
          ):
            wot_a = wo_pool.tile([128, D], F32R, tag="wot_a")
            wot_b = wo_pool.tile([64, D], F32R, tag="wot_b")
            nc.sync.dma_start(wot_a[:], wot_d.ap()[0:128, :])
            nc.sync.dma_start(wot_b[:], wot_d.ap()[128:HD, :])

            head_src = [
                (qT_a, kT_a, 0),    # h0: partitions 0-63
                (qT_a, kT_a, 64),   # h1: partitions 64-127
                (qT_b, kT_b, 0),    # h2
            ]
            for qq in range(QQ):
                q0 = qq * 512
                avs = [psum_av_pool.tile([DK + 1, 512], F32, tag=f"av{h}",
                                         name=f"av{h}_{qq}")
                       for h in range(NH)]
                for kt in range(ST):
                    m_t = mask_pool.tile([128, 512], BF16, tag="m", name="m_t")
                    nc.sync.dma_start(
                        m_t[:], mt_d.ap()[kt * 128:(kt + 1) * 128, q0:q0 + 512])
                    for h in range(NH):
                        qsrc, ksrc, p0 = head_src[h]
                        ps = psum_s_pool.tile([128, 512], F32, tag="s", name="ps_s")
                        nc.tensor.matmul(
                            ps[:],
                            ksrc[p0:p0 + DK, kt * 128:(kt + 1) * 128],
                            qsrc[p0:p0 + DK, q0:q0 + 512],
                            start=True, stop=True)
                        ex = exp_pool.tile([128, 512], BF16, tag="e", name="ex")
                        nc.scalar.activation(ex[:], ps[:], AF.Exp, scale=0.125)
                        em = expm_pool.tile([128, 512], BF16, tag="em", name="em")
                        nc.vector.tensor_tensor(em[:], ex[:], m_t[:], ALU.mult)
                        nc.tensor.matmul(
                            avs[h],
                            v_sb[:, kt, h, :],
                            em[:],
                            start=(kt == 0), stop=(kt == ST - 1))
                for h in range(NH):
                    rc = norm_pool.tile([1, 512], F32, tag="rc", name="rc")
                    nc.vector.reciprocal(rc[:], avs[h][DK:DK + 1, :])
                    bc = norm_pool.tile([64, 512], F32, tag="bc", name="bc")
                    nc.sync.dma_start(scratch_d.ap()[h, q0:q0 + 512], rc[:])
                    nc.sync.dma_start(
                        bc[:],
                        scratch_d.ap()[h, q0:q0 + 512].partition_broadcast(64))
                    if h < 2:
                        dst = attnT_a[h * 64:(h + 1) * 64, q0:q0 + 512]
                    else:
                        dst = attnT_b[0:64, q0:q0 + 512]
                    nc.vector.tensor_tensor(dst, avs[h][0:DK, :], bc[:], ALU.mult)

                # output projection for the 4 s-tiles covered by this qq block
                for st in range(qq * 4, qq * 4 + 4):
                    po = psum_o_pool.tile([128, D], F32, tag="po", name="po")
                    for (o, n) in [(0, 512), (512, 256)]:
                        nc.tensor.matmul(
                            po[:, o:o + n],
                            attnT_a[:, st * 128:(st + 1) * 128],
                            wot_a[:, o:o + n],
                            start=True, stop=False)
                        nc.tensor.matmul(
                            po[:, o:o + n],
                            attnT_b[0:64, st * 128:(st + 1) * 128],
                            wot_b[:, o:o + n],
                            start=False, stop=True)
                    ob = out_pool.tile([128, D], F32, tag="ob", name="ob")
                    nc.any.tensor_copy(ob[:], po[:])
                    nc.sync.dma_start(out_d.ap()[st * 128:(st + 1) * 128, :], ob[:])

    nc.compile()
    return nc


def make_in_maps(Q, K, V, mask, W_Q, W_K, W_V, W_O, n_cores=8):
    import ml_dtypes
    in_maps = []
    for c in range(n_cores):
        b, g = divmod(c, 4)
        hs = slice(g * HD, (g + 1) * HD)
        in_maps.append({
            "qt": np.ascontiguousarray(Q[b].T),
            "kt": np.ascontiguousarray(K[b].T),
            "vt": np.ascontiguousarray(V[b].T),
            "mt": np.ascontiguousarray(mask[b, 0].T).astype(ml_dtypes.bfloat16),
            "wqt": np.ascontiguousarray(W_Q.T[:, hs]),
            "wkt": np.ascontiguousarray(W_K.T[:, hs]),
            "wvt": np.ascontiguousarray(W_V.T[:, hs]),
            "wot": np.ascontiguousarray(W_O.T[hs, :]),
        })
    return in_maps


def combine_outputs(partials):
    b0 = partials[0] + partials[1] + partials[2] + partials[3]
    b1 = partials[4] + partials[5] + partials[6] + partials[7]
    return np.stack([b0, b1])


_NC_CACHE = {}


def _get_nc(reps=1):
    key = ("nc", reps)
    if key not in _NC_CACHE:
        _NC_CACHE[key] = build_mha_nc(S=2048, n_cores=8, norm_via_dram=True,
                                      reps=reps)
    return _NC_CACHE[key]


def kernel(Q, K, V, mask, W_Q, W_K, W_V, W_O, _reps=1):
    from concourse.bass_utils import run_bass_kernel_spmd
    nc = _get_nc(_reps)
    in_maps = make_in_maps(np.asarray(Q, np.float32), np.asarray(K, np.float32),
                           np.asarray(V, np.float32), np.asarray(mask),
                           np.asarray(W_Q, np.float32), np.asarray(W_K, np.float32),
                           np.asarray(W_V, np.float32), np.asarray(W_O, np.float32))
    res = run_bass_kernel_spmd(nc, in_maps, core_ids=list(range(8)))
    out = combine_outputs([res.results[c]["out"] for c in range(8)])
    return out.astype(np.float32)

